# revision 13
# baseline (speedup 1.0000x reference)
"""Trainium2 kernel for nn_AEDecoder: out = LeakyReLU(X @ W_sparse + bias).

The sparse edge list (400k edges over a [1639, 17000] weight matrix, 1.4%
dense) is converted on the host to a dense bf16 weight matrix — the layout the
TensorEngine consumes — with the bias folded in as an extra ones-row of X.
Each of the 8 NeuronCores gets a 2125-gene column shard of W (data-parallel
over output genes, X replicated), runs a tiled bf16 matmul with f32 PSUM
accumulation and a fused LeakyReLU epilogue, and the host concatenates the
per-core outputs.

Device schedule (per core), tuned against the ~350 GB/s shared DMA fabric:
- x_k [128,512] and W_k [128,2125] bf16 chunk pairs stream in interleaved
  (~1.9us per k), matching the PE's two-pass consumption rate.
- Pass 1 computes genes 0..1023 (8 PSUM banks: 4 batch chunks x 2 n-chunks),
  k-outer so chunk k is consumed right as it lands; pass 2 computes genes
  1024..2047 with all data resident.
- Gene tail (2048..2124, 77 genes) is computed transposed (W stationary,
  batch moving): 13 large matmuls instead of 52 tiny ones, written to a
  separate [77, 512] output that the host transposes.
- Scratch warmup matmuls at t=0 ramp the HAM clock gate to 2.4 GHz.
- LeakyReLU + PSUM->SBUF drain is a single fused ACT instruction per group.
"""

import sys

import numpy as np

for _p in ("/opt/trn_rl_repo", "/root/.axon_site/_ro/trn_rl_repo"):
    if _p not in sys.path:
        sys.path.append(_p)

import ml_dtypes

B, IN_F, OUT_F = 512, 1639, 17000
NCORES = 8
SHARD = OUT_F // NCORES      # 2125 output genes per core
K_PAD = 1664                 # 13 * 128 (1639 TF rows + 1 bias row + zero pad)
KC = K_PAD // 128            # 13 contraction chunks
NEG_SLOPE = 0.01
NTILE = 512                  # PSUM bank width in f32
NMAIN = (SHARD // NTILE) * NTILE   # 2048 genes in batch-major layout
NTAIL = SHARD - NMAIN              # 77 genes in gene-major (transposed) layout
MC = B // 128                # 4 batch chunks
WARMUP_MM = 12               # scratch matmuls to ramp the HAM clock gate

_cache: dict = {}


def _build_nc():
    import concourse.tile as tile
    from concourse import bacc, mybir

    nc = bacc.Bacc(
        "TRN2",
        target_bir_lowering=False,
        debug=False,
        num_devices=NCORES,
    )
    xT = nc.dram_tensor("xT", [K_PAD, B], mybir.dt.bfloat16, kind="ExternalInput").ap()
    w = nc.dram_tensor("w", [K_PAD, SHARD], mybir.dt.bfloat16, kind="ExternalInput").ap()
    out = nc.dram_tensor("out", [B, NMAIN], mybir.dt.bfloat16, kind="ExternalOutput").ap()
    out_t = nc.dram_tensor(
        "out_t", [NTAIL, B], mybir.dt.bfloat16, kind="ExternalOutput"
    ).ap()

    bf16 = mybir.dt.bfloat16
    f32 = mybir.dt.float32
    Lrelu = mybir.ActivationFunctionType.Lrelu

    with tile.TileContext(nc) as tc:
        with (
            tc.tile_pool(name="xp", bufs=1) as xp,
            tc.tile_pool(name="wp", bufs=1) as wp,
            tc.tile_pool(name="sp", bufs=1) as sp,
            tc.tile_pool(name="pp", bufs=8, space="PSUM") as pp,
            tc.tile_pool(name="op", bufs=6) as op,
        ):
            # Scratch warmup: keep the PE busy from t=0 so the HAM clock
            # gate ramps to 2.4 GHz before the first real matmul. Uses its
            # own SBUF tile and recycles PSUM slots from the main pool.
            scr = sp.tile([128, NTILE], bf16, tag="scr")
            nc.gpsimd.memset(scr[:], 0.0)
            scr_ps = pp.tile([128, NTILE], f32, tag="psum", name="scr_ps")
            for _ in range(WARMUP_MM):
                nc.tensor.matmul(
                    scr_ps[:], lhsT=scr[:, 0:128], rhs=scr[:], start=True, stop=True
                )

            # Deadline-ordered input stream on the shared ~350 GB/s fabric:
            # x_k plus the pass-1 half of each W chunk (cols 0:1024) first,
            # then the pass-2/tail halves (cols 1024:2125). Pass 1 needs only
            # ~5 MB before its last k-step instead of the full 8.7 MB.
            xts, was, wbs = [], [], []
            NB2 = SHARD - 2 * NTILE  # 1101: pass-2 + tail columns
            for k in range(KC):
                x_t = xp.tile([128, B], bf16, tag=f"x{k}", name=f"x_{k}")
                nc.gpsimd.dma_start(x_t[:], xT[k * 128 : (k + 1) * 128, :])
                xts.append(x_t)
                wa = wp.tile([128, 2 * NTILE], bf16, tag=f"wa{k}", name=f"wa_{k}")
                eng = nc.sync if k % 2 == 0 else nc.scalar
                eng.dma_start(wa[:], w[k * 128 : (k + 1) * 128, 0 : 2 * NTILE])
                was.append(wa)
            for k in range(KC):
                wb = wp.tile([128, NB2], bf16, tag=f"wb{k}", name=f"wb_{k}")
                eng = nc.scalar if k % 2 == 0 else nc.sync
                eng.dma_start(wb[:], w[k * 128 : (k + 1) * 128, 2 * NTILE : SHARD])
                wbs.append(wb)

            def w_slice(k, n):
                if n < 2:
                    return was[k][:, n * NTILE : (n + 1) * NTILE]
                return wbs[k][:, (n - 2) * NTILE : (n - 1) * NTILE]

            out_engs = [nc.gpsimd, nc.sync, nc.scalar]

            def emit_pass(n_lo):
                pts = {}
                for n in (n_lo, n_lo + 1):
                    for m in range(MC):
                        pts[(n, m)] = pp.tile(
                            [128, NTILE], f32, tag="psum", name=f"ps_{n}_{m}"
                        )
                for k in range(KC):
                    for n in (n_lo, n_lo + 1):
                        for m in range(MC):
                            nc.tensor.matmul(
                                pts[(n, m)][:],
                                lhsT=xts[k][:, m * 128 : (m + 1) * 128],
                                rhs=w_slice(k, n),
                                start=(k == 0),
                                stop=(k == KC - 1),
                            )
                for i, (n, m) in enumerate(
                    (n, m) for n in (n_lo, n_lo + 1) for m in range(MC)
                ):
                    ot = op.tile([128, NTILE], bf16, tag="o", name=f"o_{n}_{m}")
                    if i % 2 == 0:
                        nc.scalar.activation(
                            ot[:], pts[(n, m)][:], Lrelu, alpha=NEG_SLOPE
                        )
                    else:
                        # lrelu(x) = 0.01x + 0.99*relu(x); PSUM may only be
                        # read once per DVE instruction, so stage relu in SBUF.
                        rt = op.tile([128, NTILE], bf16, tag="r", name=f"r_{n}_{m}")
                        nc.vector.tensor_scalar(
                            rt[:],
                            pts[(n, m)][:],
                            0.0,
                            1.0 - NEG_SLOPE,
                            mybir.AluOpType.max,
                            mybir.AluOpType.mult,
                        )
                        nc.vector.scalar_tensor_tensor(
                            ot[:],
                            pts[(n, m)][:],
                            NEG_SLOPE,
                            rt[:],
                            mybir.AluOpType.mult,
                            mybir.AluOpType.add,
                        )
                    out_engs[i % 3].dma_start(
                        out[m * 128 : (m + 1) * 128, n * NTILE : (n + 1) * NTILE],
                        ot[:],
                    )

            # Pass 1: genes 0..1023, k-outer, paced with the chunk stream.
            emit_pass(0)
            # Pass 2: genes 1024..2047, all data resident.
            emit_pass(2)

            # Gene tail last, transposed: one matmul per k covers all 512
            # batch rows for the last 77 genes; its tiny epilogue is the
            # final dependency chain, overlapping pass 2's output drain.
            tail_ps = pp.tile([NTAIL, B], f32, tag="psum", name="tail_ps")
            for k in range(KC):
                nc.tensor.matmul(
                    tail_ps[:],
                    lhsT=wbs[k][:, NMAIN - 2 * NTILE : NB2],
                    rhs=xts[k][:],
                    start=(k == 0),
                    stop=(k == KC - 1),
                )
            ot_t = op.tile([NTAIL, B], bf16, tag="ot")
            nc.scalar.activation(ot_t[:], tail_ps[:], Lrelu, alpha=NEG_SLOPE)
            nc.gpsimd.dma_start(out_t[:, :], ot_t[:])

    nc.compile()
    return nc


def _prep_inputs(features, weights, bias, edge_out, edge_in):
    features = np.asarray(features, dtype=np.float32)
    weights = np.asarray(weights, dtype=np.float32)
    bias = np.asarray(bias, dtype=np.float32)
    ei = np.asarray(edge_in).astype(np.int64)
    eo = np.asarray(edge_out).astype(np.int64)

    # Sparse edge list -> dense [K_PAD, OUT_F] weight matrix, bias as row IN_F.
    W = np.zeros((K_PAD, OUT_F), dtype=np.float32)
    np.add.at(W, (ei, eo), weights)
    W[IN_F, :] = bias

    xT = np.zeros((K_PAD, B), dtype=np.float32)
    xT[:IN_F] = features.T
    xT[IN_F] = 1.0

    Wb = W.astype(ml_dtypes.bfloat16)
    xTb = np.ascontiguousarray(xT.astype(ml_dtypes.bfloat16))
    return [
        {
            "xT": xTb,
            "w": np.ascontiguousarray(Wb[:, c * SHARD : (c + 1) * SHARD]),
        }
        for c in range(NCORES)
    ]


def _assemble(results):
    cols = []
    for c in range(NCORES):
        cols.append(results[c]["out"].astype(np.float32))
        cols.append(results[c]["out_t"].T.astype(np.float32))
    return np.concatenate(cols, axis=1)


def kernel(features, weights, bias, edge_out, edge_in):
    from concourse import bass_utils

    in_maps = _prep_inputs(features, weights, bias, edge_out, edge_in)
    if "nc" not in _cache:
        _cache["nc"] = _build_nc()
    nc = _cache["nc"]
    res = bass_utils.run_bass_kernel_spmd(nc, in_maps, core_ids=list(range(NCORES)))
    return _assemble(res.results)


# revision 15
# speedup vs baseline: 1.0967x; 1.0967x over previous
"""Trainium2 kernel for nn_AEDecoder: out = LeakyReLU(X @ W_sparse + bias).

The sparse edge list (400k edges over a [1639, 17000] weight matrix, 1.4%
dense) is converted on the host to a dense bf16 weight matrix — the layout the
TensorEngine consumes — with the bias folded in as an extra ones-row of X.
Each of the 8 NeuronCores gets a 2125-gene column shard of W (data-parallel
over output genes, X replicated), runs a tiled bf16 matmul with f32 PSUM
accumulation and a fused LeakyReLU epilogue, and the host concatenates the
per-core outputs.

Device schedule (per core), tuned against the ~350 GB/s shared DMA fabric:
- x_k [128,512] and W_k [128,2125] bf16 chunk pairs stream in interleaved
  (~1.9us per k), matching the PE's two-pass consumption rate.
- Pass 1 computes genes 0..1023 (8 PSUM banks: 4 batch chunks x 2 n-chunks),
  k-outer so chunk k is consumed right as it lands; pass 2 computes genes
  1024..2047 with all data resident.
- Gene tail (2048..2124, 77 genes) is computed transposed (W stationary,
  batch moving): 13 large matmuls instead of 52 tiny ones, written to a
  separate [77, 512] output that the host transposes.
- Scratch warmup matmuls at t=0 ramp the HAM clock gate to 2.4 GHz.
- LeakyReLU + PSUM->SBUF drain is a single fused ACT instruction per group.
"""

import sys

import numpy as np

for _p in ("/opt/trn_rl_repo", "/root/.axon_site/_ro/trn_rl_repo"):
    if _p not in sys.path:
        sys.path.append(_p)

import ml_dtypes

B, IN_F, OUT_F = 512, 1639, 17000
NCORES = 8
SHARD = OUT_F // NCORES      # 2125 output genes per core
K_PAD = 1664                 # 13 * 128 (1639 TF rows + 1 bias row + zero pad)
KC = K_PAD // 128            # 13 contraction chunks
NEG_SLOPE = 0.01
NTILE = 512                  # PSUM bank width in f32
NMAIN = (SHARD // NTILE) * NTILE   # 2048 genes in batch-major layout
NTAIL = SHARD - NMAIN              # 77 genes in gene-major (transposed) layout
MC = B // 128                # 4 batch chunks
WARMUP_MM = 12               # scratch matmuls to ramp the HAM clock gate

_cache: dict = {}


def _build_nc():
    import concourse.tile as tile
    from concourse import bacc, mybir
    nc = bacc.Bacc(
        "TRN2",
        target_bir_lowering=False,
        debug=False,
        num_devices=NCORES,
    )
    xT = nc.dram_tensor("xT", [K_PAD, B], mybir.dt.bfloat16, kind="ExternalInput").ap()
    w = nc.dram_tensor("w", [K_PAD, SHARD], mybir.dt.bfloat16, kind="ExternalInput").ap()
    out = nc.dram_tensor("out", [B, NMAIN], mybir.dt.bfloat16, kind="ExternalOutput").ap()
    out_t = nc.dram_tensor(
        "out_t", [NTAIL, B], mybir.dt.bfloat16, kind="ExternalOutput"
    ).ap()

    bf16 = mybir.dt.bfloat16
    f32 = mybir.dt.float32
    Lrelu = mybir.ActivationFunctionType.Lrelu

    with tile.TileContext(nc) as tc:
        with (
            tc.tile_pool(name="xp", bufs=1) as xp,
            tc.tile_pool(name="wp", bufs=1) as wp,
            tc.tile_pool(name="sp", bufs=1) as sp,
            tc.tile_pool(name="pp", bufs=8, space="PSUM") as pp,
            tc.tile_pool(name="op", bufs=6) as op,
        ):
            # Scratch warmup: keep the PE busy from t=0 so the HAM clock
            # gate ramps to 2.4 GHz before the first real matmul. The scratch
            # operand is a raw (untracked, uninitialized) SBUF tensor so the
            # warmup needs no memset and no waits at all.
            scr = nc.alloc_sbuf_tensor("scr_raw", [128, NTILE], bf16).ap()
            scr_ps = pp.tile([128, NTILE], f32, tag="psum", name="scr_ps")
            for _ in range(WARMUP_MM):
                nc.tensor.matmul(
                    scr_ps[:], lhsT=scr[:, 0:128], rhs=scr[:], start=True, stop=True
                )

            # Interleaved x_k / W_k chunk stream. x on GpSimd's queue, W
            # alternating Sync/Scalar queues; the fabric fair-shares, so the
            # k-th pair lands about (k+1)*1.9us in.
            xts, wts = [], []
            for k in range(KC):
                x_t = xp.tile([128, B], bf16, tag=f"x{k}", name=f"x_{k}")
                nc.gpsimd.dma_start(x_t[:], xT[k * 128 : (k + 1) * 128, :])
                xts.append(x_t)
                w_t = wp.tile([128, SHARD], bf16, tag=f"w{k}", name=f"w_{k}")
                eng = nc.sync if k % 2 == 0 else nc.scalar
                eng.dma_start(w_t[:], w[k * 128 : (k + 1) * 128, :])
                wts.append(w_t)

            def w_slice(k, n):
                return wts[k][:, n * NTILE : (n + 1) * NTILE]

            out_engs = [nc.gpsimd, nc.sync, nc.scalar]

            def emit_pass(n_lo):
                pts = {}
                for n in (n_lo, n_lo + 1):
                    for m in range(MC):
                        pts[(n, m)] = pp.tile(
                            [128, NTILE], f32, tag="psum", name=f"ps_{n}_{m}"
                        )
                for k in range(KC):
                    for n in (n_lo, n_lo + 1):
                        for m in range(MC):
                            nc.tensor.matmul(
                                pts[(n, m)][:],
                                lhsT=xts[k][:, m * 128 : (m + 1) * 128],
                                rhs=w_slice(k, n),
                                start=(k == 0),
                                stop=(k == KC - 1),
                            )
                for i, (n, m) in enumerate(
                    (n, m) for n in (n_lo, n_lo + 1) for m in range(MC)
                ):
                    ot = op.tile([128, NTILE], bf16, tag="o", name=f"o_{n}_{m}")
                    if i % 2 == 0:
                        nc.scalar.activation(
                            ot[:], pts[(n, m)][:], Lrelu, alpha=NEG_SLOPE
                        )
                    else:
                        # lrelu(x) = 0.01x + 0.99*relu(x); PSUM may only be
                        # read once per DVE instruction, so stage relu in SBUF.
                        rt = op.tile([128, NTILE], bf16, tag="r", name=f"r_{n}_{m}")
                        nc.vector.tensor_scalar(
                            rt[:],
                            pts[(n, m)][:],
                            0.0,
                            1.0 - NEG_SLOPE,
                            mybir.AluOpType.max,
                            mybir.AluOpType.mult,
                        )
                        nc.vector.scalar_tensor_tensor(
                            ot[:],
                            pts[(n, m)][:],
                            NEG_SLOPE,
                            rt[:],
                            mybir.AluOpType.mult,
                            mybir.AluOpType.add,
                        )
                    out_engs[i % 3].dma_start(
                        out[m * 128 : (m + 1) * 128, n * NTILE : (n + 1) * NTILE],
                        ot[:],
                    )

            # Pass 1: genes 0..1023, k-outer, paced with the chunk stream.
            emit_pass(0)
            # Pass 2: genes 1024..2047, all data resident.
            emit_pass(2)

            # Gene tail last, transposed: one matmul per k covers all 512
            # batch rows for the last 77 genes; its tiny epilogue is the
            # final dependency chain, overlapping pass 2's output drain.
            tail_ps = pp.tile([NTAIL, B], f32, tag="psum", name="tail_ps")
            for k in range(KC):
                nc.tensor.matmul(
                    tail_ps[:],
                    lhsT=wts[k][:, NMAIN:SHARD],
                    rhs=xts[k][:],
                    start=(k == 0),
                    stop=(k == KC - 1),
                )
            ot_t = op.tile([NTAIL, B], bf16, tag="ot")
            nc.scalar.activation(ot_t[:], tail_ps[:], Lrelu, alpha=NEG_SLOPE)
            nc.gpsimd.dma_start(out_t[:, :], ot_t[:])

    nc.compile()
    return nc


def _prep_inputs(features, weights, bias, edge_out, edge_in):
    features = np.asarray(features, dtype=np.float32)
    weights = np.asarray(weights, dtype=np.float32)
    bias = np.asarray(bias, dtype=np.float32)
    ei = np.asarray(edge_in).astype(np.int64)
    eo = np.asarray(edge_out).astype(np.int64)

    # Sparse edge list -> dense [K_PAD, OUT_F] weight matrix, bias as row IN_F.
    W = np.zeros((K_PAD, OUT_F), dtype=np.float32)
    np.add.at(W, (ei, eo), weights)
    W[IN_F, :] = bias

    xT = np.zeros((K_PAD, B), dtype=np.float32)
    xT[:IN_F] = features.T
    xT[IN_F] = 1.0

    Wb = W.astype(ml_dtypes.bfloat16)
    xTb = np.ascontiguousarray(xT.astype(ml_dtypes.bfloat16))
    return [
        {
            "xT": xTb,
            "w": np.ascontiguousarray(Wb[:, c * SHARD : (c + 1) * SHARD]),
        }
        for c in range(NCORES)
    ]


def _assemble(results):
    cols = []
    for c in range(NCORES):
        cols.append(results[c]["out"].astype(np.float32))
        cols.append(results[c]["out_t"].T.astype(np.float32))
    return np.concatenate(cols, axis=1)


def kernel(features, weights, bias, edge_out, edge_in):
    from concourse import bass_utils

    in_maps = _prep_inputs(features, weights, bias, edge_out, edge_in)
    if "nc" not in _cache:
        _cache["nc"] = _build_nc()
    nc = _cache["nc"]
    res = bass_utils.run_bass_kernel_spmd(nc, in_maps, core_ids=list(range(NCORES)))
    return _assemble(res.results)
